# revision 29
# baseline (speedup 1.0000x reference)
"""Trainium2 Bass kernel for nn_CVFRLayer (recurrent attractor scan).

x_{t+1} = (1-dt)*x_t + nl(x_t) @ B' + z_t,   nl(x) = x^2/(gamma+x^2)
  B' = dt*(A@(I-P) + P).T  (P block-diagonal projector, computed host-side O(n^2))
  z_t = noise_t @ (sqrt(dt)*eps*G.T)

Strategy: pure data parallel over 8 NeuronCores, 64 batch rows per core.
State kept in a "folded" layout [128, 1024]: partitions 0-63 hold features
0-1023, partitions 64-127 hold features 1024-2047 for the same 64 batch rows.

Scan matmul: fp16, column-tiled pairs (tile_position (0,0)/(0,64)) so two M=64
matmuls occupy the full 128x128 PE array concurrently (~2x effective
throughput; fp16 full-array streaming is the roofline for this shape).

Noise projection: fp8 e4m3 with perf_mode=DoubleRow (2 fp8 weights/cell,
K=256 per matmul) — ~1.5x over fp16, and it is half of all FLOPs. G.T is
pre-scaled by S=512 on the host so its ~4e-4-magnitude entries land in e4m3's
normal range; the 1/S is folded into the psum->sbuf copy-out. CPU simulation
of this quantization: rel err 2.9e-3 (vs 1.5e-4 all-fp16), well under the
2e-2 gate. fp8 for the scan matmul too would be 1.9e-2 — too close; and DR
is mutually exclusive with column tiling, so it would not be faster anyway.

Per-step critical chain (add -> nl -> fT transpose) is hidden by (a) splitting
the fT xbar-transpose DMAs per 512-col chunk, so k-tiles {0-3,8-11} of the
next step's stationary operand are ready after chunk-0's epilogue, (b)
ordering each psum accumulation's k-loop early-tiles-first, and (c)
interleaving the DR noise matmuls after each step's scan matmuls.
"""

import math
import sys

if "/opt/trn_rl_repo" not in sys.path:
    sys.path.insert(0, "/opt/trn_rl_repo")

import numpy as np

SIZE = 2048
N_CLASSES = 16
STEPS = 100
DT = 0.03
GAMMA = 0.125
BETA = 1.0
EIG = 1.0
EPSILON = 0.1

N_CORES = 8
BPC = 64  # batch rows per core
HALF = SIZE // 2  # folded free dim
KT = SIZE // 128  # 16 contraction tiles
KP = KT // 2  # 8 DoubleRow k-pairs
A_COEF = 1.0 - DT
CHUNKS = [(0, 512), (512, 1024)]  # folded-col chunks per step
S_GT = 512.0  # fp8 scale for gt (entries ~4e-4 -> ~0.2)
INV_S = 1.0 / S_GT
LEAD = 2  # noise supertiles produced ahead of consumption

_cache = {}


def _build(steps):
    import concourse.bacc as bacc
    import concourse.mybir as mybir
    import concourse.tile as tile

    f8 = mybir.dt.float8e4
    f16 = mybir.dt.float16
    f32 = mybir.dt.float32
    AF = mybir.ActivationFunctionType
    OP = mybir.AluOpType
    DR = mybir.MatmulPerfMode.DoubleRow

    n_st = steps // 2  # noise supertiles (2 steps each)
    assert steps % 2 == 0

    nc = bacc.Bacc("TRN2", target_bir_lowering=False, debug=False, num_devices=N_CORES)
    x0_d = nc.declare_dram_parameter("x0", [128, HALF], f32, isOutput=False)
    bt_d = nc.declare_dram_parameter("bt", [SIZE, SIZE], f16, isOutput=False)
    gt_d = nc.declare_dram_parameter("gt", [SIZE, SIZE], f8, isOutput=False)
    nt_d = nc.declare_dram_parameter("nt", [SIZE, steps * BPC], f8, isOutput=False)
    out_d = nc.declare_dram_parameter("out", [128, HALF], f32, isOutput=True)

    # 3D views with the 128-partition dim first: [(k p) n -> p k n]
    bt_v = bt_d.rearrange("(k p) n -> p k n", p=128)
    gt_v = gt_d.rearrange("(k p) n -> p k n", p=128)
    nt_v = nt_d.rearrange("(k p) n -> p k n", p=128)

    with tile.TileContext(nc) as tc:
        with (
            tc.tile_pool(name="const", bufs=1) as constp,
            tc.tile_pool(name="state", bufs=1) as statep,
            tc.tile_pool(name="f16t", bufs=2) as f16p,
            tc.tile_pool(name="fT", bufs=2) as fTp,
            tc.tile_pool(name="zu", bufs=2) as zup,
            tc.tile_pool(name="zf", bufs=6) as zfp,
            tc.tile_pool(name="nt", bufs=4) as ntp,
            tc.tile_pool(name="scr", bufs=2) as scrp,
            tc.tile_pool(name="sps", bufs=4, space="PSUM") as spsp,
            tc.tile_pool(name="nps", bufs=3, space="PSUM") as npsp,
        ):
            # ---- persistent tiles ----
            bt = constp.tile([128, KT, SIZE], f16, tag="bt")
            gt = constp.tile([128, KT, SIZE], f8, tag="gt")
            x = statep.tile([128, HALF], f32, tag="x")
            w = statep.tile([128, HALF], f32, tag="w")

            def nl_chunk(src_ap, dst_ap, wd):
                """dst(f16) = nl(src) for a [128, wd] slice.

                All on DVE: the whole add->nl chain then runs in-order on one
                engine with no cross-engine semaphore hops (the chain gates the
                next step's stationary operand, so its latency is critical).
                """
                s = scrp.tile([128, 512], f32, tag="s")
                d = scrp.tile([128, 512], f32, tag="d")
                r = scrp.tile([128, 512], f32, tag="r")
                nc.vector.tensor_mul(s[:, :wd], src_ap, src_ap)
                nc.vector.tensor_scalar_add(d[:, :wd], s[:, :wd], GAMMA)
                nc.vector.reciprocal_approx_fast(r[:, :wd], d[:, :wd])
                nc.vector.tensor_scalar(dst_ap, r[:, :wd], -GAMMA, 1.0, OP.mult, OP.add)

            def transposes_chunk(f16t, ci, c0, c1):
                """Transpose chunk [c0:c1] of both state halves into two fresh
                fT tiles (one per half). Separate tiles per (chunk, half) —
                NOT slices of one tile — so the 4 transpose DMAs carry no WAW
                relation at all (tile-granular WAW tracking otherwise
                serializes them across queues via semaphore-reset barriers).
                The two DMAs go to different HWDGE queues (SP + ACT)."""
                lo = fTp.tile([128, 4, 64], f16, tag=f"fTlo{ci}")
                hi = fTp.tile([128, 4, 64], f16, tag=f"fThi{ci}")
                nc.sync.dma_start_transpose(lo[:], f16t[0:64, c0:c1])
                nc.scalar.dma_start_transpose(hi[:], f16t[64:128, c0:c1])
                return lo, hi

            def fT_ap(fts, k):
                """Stationary operand for contraction tile k from the 4-tile
                set fts = {ci: (lo, hi)}. k 0-3 -> chunk0 lo, 4-7 -> chunk1 lo,
                8-11 -> chunk0 hi, 12-15 -> chunk1 hi."""
                lo, hi = fts[(k % 8) // 4]
                return (lo if k < 8 else hi)[:, k % 4, :]

            nt_tiles = {}
            nt_next = [LEAD + 1]  # first supertile not explicitly prefetched

            def nt_prefetch(st, engine=None):
                if st < n_st:
                    ntt = ntp.tile([128, KT, 128], f8, tag="nt")
                    eng = engine if engine is not None else nc.gpsimd
                    eng.dma_start(ntt[:], nt_v[:, :, st * 128 : (st + 1) * 128])
                    nt_tiles[st] = ntt

            def nt_prefetch_next():
                if nt_next[0] < n_st:
                    nt_prefetch(nt_next[0])
                    nt_next[0] += 1

            # initial DMA order (single ACT queue, 1B/elem for fp8):
            # x0 first (unblocks f(x0)); nt for the lead supertiles; gt cols
            # 0-1024 (first noise halves) then 1024-2048; bt cols for scan
            # chunk 0 (n=0,2) then chunk 1 (n=1,3).
            nc.scalar.dma_start(x[:], x0_d[:])
            for st0 in range(min(LEAD + 1, n_st)):
                nt_prefetch(st0, engine=nc.scalar)
            for n in (0, 1, 2, 3):
                nc.scalar.dma_start(
                    gt[:, :, n * 512 : (n + 1) * 512],
                    gt_v[:, :, n * 512 : (n + 1) * 512],
                )
            for n in (0, 2, 1, 3):
                nc.scalar.dma_start(
                    bt[:, :, n * 512 : (n + 1) * 512],
                    bt_v[:, :, n * 512 : (n + 1) * 512],
                )

            # ---- initial f(x0) ----
            f16t0 = f16p.tile([128, HALF], f16, tag="f16t")
            fT_cur = {}
            for ci, (c0, c1) in enumerate(CHUNKS):
                nl_chunk(x[:, c0:c1], f16t0[:, c0:c1], c1 - c0)
                fT_cur[ci] = transposes_chunk(f16t0, ci, c0, c1)

            zf_tiles = {}
            zu_tiles = {}

            def noise_half(st, half):
                """Half a noise supertile: feature chunks [2*half, 2*half+2).

                Supertile st projects noise rows [128*st, 128*st+128) -> z for
                steps 2st, 2st+1, via fp8 DoubleRow matmuls (K=256 per MM).
                """
                if half == 0:
                    ntt = nt_tiles.pop(st)
                    zu = zup.tile([128, SIZE], f16, tag="zu")
                    zu_tiles[st] = (ntt, zu)
                    nt_prefetch_next()
                else:
                    ntt, zu = zu_tiles.pop(st)
                for n in (2 * half, 2 * half + 1):
                    ps = npsp.tile([128, 512], f32, tag="nps")
                    for k in range(KP):
                        nc.tensor.matmul(
                            ps[:],
                            ntt[:, 2 * k : 2 * k + 2, :],
                            gt[:, 2 * k : 2 * k + 2, n * 512 : (n + 1) * 512],
                            start=(k == 0),
                            stop=(k == KP - 1),
                            perf_mode=DR,
                        )
                    # copy out (fp32 psum -> fp16 sbuf) with the 1/S rescale;
                    # alternate engines
                    if n % 2 == 0:
                        nc.scalar.activation(
                            zu[:, n * 512 : (n + 1) * 512], ps[:], AF.Copy, scale=INV_S
                        )
                    else:
                        nc.vector.tensor_scalar_mul(
                            zu[:, n * 512 : (n + 1) * 512], ps[:], INV_S
                        )
                if half == 1:
                    # fold into per-step layout via SBUF->SBUF DMA on the
                    # GpSimd queue (same queue as the w-update that reads zf,
                    # and off the ACT queue that carries critical transposes)
                    for h in (0, 1):
                        t = 2 * st + h
                        zf = zfp.tile([128, HALF], f16, tag="zf")
                        nc.gpsimd.dma_start(zf[0:64, :], zu[h * 64 : h * 64 + 64, 0:HALF])
                        nc.gpsimd.dma_start(
                            zf[64:128, :], zu[h * 64 : h * 64 + 64, HALF:SIZE]
                        )
                        zf_tiles[t] = zf

            # lead: LEAD supertiles (z for steps 0..2*LEAD-1) before the scan;
            # all h0 halves first (they only need gt cols 0-1024, which land
            # before cols 1024-2048 in the DMA queue)
            for st0 in range(min(LEAD, n_st)):
                noise_half(st0, 0)
            for st0 in range(min(LEAD, n_st)):
                noise_half(st0, 1)

            # k-loop order: tiles whose fT comes from chunk 0's transposes
            # first — they are ready one chunk-epilogue earlier than chunk 1's
            K_ORDER = [0, 1, 2, 3, 8, 9, 10, 11, 4, 5, 6, 7, 12, 13, 14, 15]

            # ---- the scan ----
            for t in range(steps):
                zf = zf_tiles.pop(t)
                # w = (1-dt)*x + z_t, split per chunk: each half depends only
                # on that chunk's add from the previous step, so the c0 half
                # schedules well before the critical c1 add->nl chain
                for c0, c1 in CHUNKS:
                    nc.vector.scalar_tensor_tensor(
                        w[:, c0:c1], x[:, c0:c1], A_COEF, zf[:, c0:c1],
                        OP.mult, OP.add,
                    )
                f16t = f16p.tile([128, HALF], f16, tag="f16t")
                fT_next = {}
                for ci, (c0, c1) in enumerate(CHUNKS):
                    wd = c1 - c0
                    ps = spsp.tile([128, 512], f32, tag="sps")
                    for idx, k in enumerate(K_ORDER):
                        fTk = fT_ap(fT_cur, k)
                        nc.tensor.matmul(
                            ps[0:64, :wd],
                            fTk,
                            bt[:, k, c0:c1],
                            start=(idx == 0),
                            stop=(idx == KT - 1),
                            tile_position=(0, 0),
                        )
                        nc.tensor.matmul(
                            ps[64:128, :wd],
                            fTk,
                            bt[:, k, HALF + c0 : HALF + c1],
                            start=(idx == 0),
                            stop=(idx == KT - 1),
                            tile_position=(0, 64),
                        )
                    # x_new = y + w
                    nc.vector.tensor_add(x[:, c0:c1], ps[:, :wd], w[:, c0:c1])
                    nl_chunk(x[:, c0:c1], f16t[:, c0:c1], wd)
                    fT_next[ci] = transposes_chunk(f16t, ci, c0, c1)
                fT_cur = fT_next
                # interleave noise production at consumption rate:
                # half a supertile after every scan step, LEAD supertiles ahead
                st, half = t // 2 + LEAD, t % 2
                if st < n_st:
                    noise_half(st, half)

            nc.scalar.dma_start(out_d[:], x[:])

    nc.compile()
    return nc


def _prepare_host(x, A, G, noise, steps):
    """Host-side O(n^2) weight prep + per-core input shards."""
    import ml_dtypes

    E4M3 = ml_dtypes.float8_e4m3

    block = SIZE // N_CLASSES
    P = np.zeros((SIZE, SIZE), dtype=np.float32)
    for c in range(N_CLASSES):
        P[c * block : (c + 1) * block, c * block : (c + 1) * block] = 1.0 / block
    Ab = A.reshape(SIZE, N_CLASSES, block).mean(axis=2)
    A_P = np.repeat(Ab, block, axis=1)  # A @ P
    M0 = A - A_P + EIG * P  # A @ (I-P) + P
    bt_np = np.ascontiguousarray((DT * BETA) * M0.T).astype(np.float16)
    gt_np = np.ascontiguousarray((S_GT * math.sqrt(DT) * EPSILON) * G.T).astype(E4M3)

    in_maps = []
    for c in range(N_CORES):
        xs = x[c * BPC : (c + 1) * BPC]
        x0f = np.concatenate([xs[:, :HALF], xs[:, HALF:]], axis=0)
        x0f = np.ascontiguousarray(x0f, dtype=np.float32)
        nsh = noise[:steps, c * BPC : (c + 1) * BPC, :].reshape(steps * BPC, SIZE)
        nt_np = np.ascontiguousarray(nsh.astype(E4M3).T)
        in_maps.append({"x0": x0f, "bt": bt_np, "gt": gt_np, "nt": nt_np})
    return in_maps


def _run(in_maps, steps, trace=False):
    from concourse.bass_utils import run_bass_kernel_spmd

    key = steps
    if key not in _cache:
        _cache[key] = _build(steps)
    nc = _cache[key]
    res = run_bass_kernel_spmd(nc, in_maps, list(range(N_CORES)), trace=trace)
    outs = []
    for c in range(N_CORES):
        of = res.results[c]["out"]
        outs.append(np.concatenate([of[0:64, :], of[64:128, :]], axis=1))
    return np.concatenate(outs, axis=0).astype(np.float32), res


def kernel(x, A, G, noise):
    x = np.asarray(x, dtype=np.float32)
    A = np.asarray(A, dtype=np.float32)
    G = np.asarray(G, dtype=np.float32)
    noise = np.asarray(noise, dtype=np.float32)
    in_maps = _prepare_host(x, A, G, noise, STEPS)
    out, _ = _run(in_maps, STEPS)
    return out


# revision 35
# speedup vs baseline: 1.0150x; 1.0150x over previous
"""Trainium2 Bass kernel for nn_CVFRLayer (recurrent attractor scan).

x_{t+1} = (1-dt)*x_t + nl(x_t) @ B' + z_t,   nl(x) = x^2/(gamma+x^2)
  B' = dt*(A@(I-P) + P).T  (P block-diagonal projector, computed host-side O(n^2))
  z_t = noise_t @ (sqrt(dt)*eps*G.T)

Strategy: pure data parallel over 8 NeuronCores, 64 batch rows per core.
State kept in a "folded" layout [128, 1024]: partitions 0-63 hold features
0-1023, partitions 64-127 hold features 1024-2047 for the same 64 batch rows.

Scan matmul: fp16, column-tiled pairs (tile_position (0,0)/(0,64)) so two M=64
matmuls occupy the full 128x128 PE array concurrently (~2x effective
throughput; fp16 full-array streaming is the roofline for this shape).

Noise projection: fp8 e4m3 with perf_mode=DoubleRow (2 fp8 weights/cell,
K=256 per matmul) — ~1.5x over fp16, and it is half of all FLOPs. G.T is
pre-scaled by S=512 on the host so its ~4e-4-magnitude entries land in e4m3's
normal range; the 1/S is folded into the psum->sbuf copy-out. CPU simulation
of this quantization: rel err 2.9e-3 (vs 1.5e-4 all-fp16), well under the
2e-2 gate. fp8 for the scan matmul too would be 1.9e-2 — too close; and DR
is mutually exclusive with column tiling, so it would not be faster anyway.

Per-step critical chain (add -> nl -> fT transpose) is hidden by (a) splitting
the fT xbar-transpose DMAs per 512-col chunk, so k-tiles {0-3,8-11} of the
next step's stationary operand are ready after chunk-0's epilogue, (b)
ordering each psum accumulation's k-loop early-tiles-first, and (c)
interleaving the DR noise matmuls after each step's scan matmuls.
"""

import math
import sys

if "/opt/trn_rl_repo" not in sys.path:
    sys.path.insert(0, "/opt/trn_rl_repo")

import numpy as np

SIZE = 2048
N_CLASSES = 16
STEPS = 100
DT = 0.03
GAMMA = 0.125
BETA = 1.0
EIG = 1.0
EPSILON = 0.1

N_CORES = 8
BPC = 64  # batch rows per core
HALF = SIZE // 2  # folded free dim
KT = SIZE // 128  # 16 contraction tiles
KP = KT // 2  # 8 DoubleRow k-pairs
A_COEF = 1.0 - DT
CHUNKS = [(0, 512), (512, 1024)]  # folded-col chunks per step
S_GT = 512.0  # fp8 scale for gt (entries ~4e-4 -> ~0.2)
INV_S = 1.0 / S_GT
LEAD = 2  # noise supertiles produced ahead of consumption

_cache = {}


def _build(steps):
    import concourse.bacc as bacc
    import concourse.mybir as mybir
    import concourse.tile as tile

    f8 = mybir.dt.float8e4
    f16 = mybir.dt.float16
    f32 = mybir.dt.float32
    AF = mybir.ActivationFunctionType
    OP = mybir.AluOpType
    DR = mybir.MatmulPerfMode.DoubleRow

    n_st = steps // 2  # noise supertiles (2 steps each)
    assert steps % 2 == 0

    nc = bacc.Bacc("TRN2", target_bir_lowering=False, debug=False, num_devices=N_CORES)
    x0_d = nc.declare_dram_parameter("x0", [128, HALF], f32, isOutput=False)
    bt_d = nc.declare_dram_parameter("bt", [SIZE, SIZE], f16, isOutput=False)
    gt_d = nc.declare_dram_parameter("gt", [SIZE, SIZE], f8, isOutput=False)
    nt_d = nc.declare_dram_parameter("nt", [SIZE, steps * BPC], f8, isOutput=False)
    out_d = nc.declare_dram_parameter("out", [128, HALF], f32, isOutput=True)

    # 3D views with the 128-partition dim first: [(k p) n -> p k n]
    bt_v = bt_d.rearrange("(k p) n -> p k n", p=128)
    gt_v = gt_d.rearrange("(k p) n -> p k n", p=128)
    nt_v = nt_d.rearrange("(k p) n -> p k n", p=128)

    with tile.TileContext(nc) as tc:
        with (
            tc.tile_pool(name="const", bufs=1) as constp,
            tc.tile_pool(name="state", bufs=1) as statep,
            tc.tile_pool(name="f16t", bufs=2) as f16p,
            tc.tile_pool(name="fT", bufs=2) as fTp,
            tc.tile_pool(name="zu", bufs=2) as zup,
            tc.tile_pool(name="zf", bufs=6) as zfp,
            tc.tile_pool(name="nt", bufs=4) as ntp,
            tc.tile_pool(name="scr", bufs=2) as scrp,
            tc.tile_pool(name="sps", bufs=4, space="PSUM") as spsp,
            tc.tile_pool(name="nps", bufs=3, space="PSUM") as npsp,
        ):
            # ---- persistent tiles ----
            bt = constp.tile([128, KT, SIZE], f16, tag="bt")
            gt = constp.tile([128, KT, SIZE], f8, tag="gt")
            x = statep.tile([128, HALF], f32, tag="x")
            w = statep.tile([128, HALF], f32, tag="w")

            def nl_chunk(src_ap, dst_ap, wd):
                """dst(f16) = nl(src) for a [128, wd] slice.

                All on DVE: the whole add->nl chain then runs in-order on one
                engine with no cross-engine semaphore hops (the chain gates the
                next step's stationary operand, so its latency is critical).
                """
                s = scrp.tile([128, 512], f32, tag="s")
                d = scrp.tile([128, 512], f32, tag="d")
                r = scrp.tile([128, 512], f32, tag="r")
                nc.vector.tensor_mul(s[:, :wd], src_ap, src_ap)
                nc.vector.tensor_scalar_add(d[:, :wd], s[:, :wd], GAMMA)
                nc.vector.reciprocal_approx_fast(r[:, :wd], d[:, :wd])
                nc.vector.tensor_scalar(dst_ap, r[:, :wd], -GAMMA, 1.0, OP.mult, OP.add)

            def transposes(f16t):
                """One merged xbar transpose per state half into two SEPARATE
                fT tiles (k-tiles 0-7 and 8-15). Separate tiles — not slices
                of one tile — so the two DMAs carry no WAW relation and run
                concurrently on the two HWDGE queues instead of serializing
                through a tile-WAW semaphore-reset barrier (~2.8us observed).
                Exactly two transposes per step (more, spread over two queues,
                was observed numerically flaky on HW)."""
                lo = fTp.tile([128, 8, 64], f16, tag="fTlo")
                hi = fTp.tile([128, 8, 64], f16, tag="fThi")
                nc.sync.dma_start_transpose(lo[:], f16t[0:64, :])
                nc.scalar.dma_start_transpose(hi[:], f16t[64:128, :])
                return lo, hi

            def fT_ap(fts, k):
                """Stationary operand for contraction tile k: k 0-7 from the
                lower-partition (features 0-1023) tile, 8-15 from the upper."""
                lo, hi = fts
                return (lo if k < 8 else hi)[:, k % 8, :]

            nt_tiles = {}
            nt_next = [LEAD + 1]  # first supertile not explicitly prefetched

            def nt_prefetch(st, engine=None):
                if st < n_st:
                    ntt = ntp.tile([128, KT, 128], f8, tag="nt")
                    eng = engine if engine is not None else nc.gpsimd
                    eng.dma_start(ntt[:], nt_v[:, :, st * 128 : (st + 1) * 128])
                    nt_tiles[st] = ntt

            def nt_prefetch_next():
                if nt_next[0] < n_st:
                    nt_prefetch(nt_next[0])
                    nt_next[0] += 1

            # initial DMA order (single ACT queue, 1B/elem for fp8):
            # x0 first (unblocks f(x0)); nt for the lead supertiles; gt cols
            # 0-1024 (first noise halves) then 1024-2048; bt cols for scan
            # chunk 0 (n=0,2) then chunk 1 (n=1,3).
            nc.scalar.dma_start(x[:], x0_d[:])
            for st0 in range(min(LEAD + 1, n_st)):
                nt_prefetch(st0, engine=nc.scalar)
            for n in (0, 1, 2, 3):
                nc.scalar.dma_start(
                    gt[:, :, n * 512 : (n + 1) * 512],
                    gt_v[:, :, n * 512 : (n + 1) * 512],
                )
            for n in (0, 2, 1, 3):
                nc.scalar.dma_start(
                    bt[:, :, n * 512 : (n + 1) * 512],
                    bt_v[:, :, n * 512 : (n + 1) * 512],
                )

            # ---- initial f(x0) ----
            f16t0 = f16p.tile([128, HALF], f16, tag="f16t")
            for c0, c1 in CHUNKS:
                nl_chunk(x[:, c0:c1], f16t0[:, c0:c1], c1 - c0)
            fT_cur = transposes(f16t0)

            zf_tiles = {}
            zu_tiles = {}

            def noise_half(st, half):
                """Half a noise supertile: feature chunks [2*half, 2*half+2).

                Supertile st projects noise rows [128*st, 128*st+128) -> z for
                steps 2st, 2st+1, via fp8 DoubleRow matmuls (K=256 per MM).
                """
                if half == 0:
                    ntt = nt_tiles.pop(st)
                    zu = zup.tile([128, SIZE], f16, tag="zu")
                    zu_tiles[st] = (ntt, zu)
                    nt_prefetch_next()
                else:
                    ntt, zu = zu_tiles.pop(st)
                for n in (2 * half, 2 * half + 1):
                    ps = npsp.tile([128, 512], f32, tag="nps")
                    for k in range(KP):
                        nc.tensor.matmul(
                            ps[:],
                            ntt[:, 2 * k : 2 * k + 2, :],
                            gt[:, 2 * k : 2 * k + 2, n * 512 : (n + 1) * 512],
                            start=(k == 0),
                            stop=(k == KP - 1),
                            perf_mode=DR,
                        )
                    # copy out (fp32 psum -> fp16 sbuf) with the 1/S rescale;
                    # alternate engines
                    if n % 2 == 0:
                        nc.scalar.activation(
                            zu[:, n * 512 : (n + 1) * 512], ps[:], AF.Copy, scale=INV_S
                        )
                    else:
                        nc.vector.tensor_scalar_mul(
                            zu[:, n * 512 : (n + 1) * 512], ps[:], INV_S
                        )
                if half == 1:
                    # fold into per-step layout via SBUF->SBUF DMA on the
                    # GpSimd queue (same queue as the w-update that reads zf,
                    # and off the ACT queue that carries critical transposes)
                    for h in (0, 1):
                        t = 2 * st + h
                        zf = zfp.tile([128, HALF], f16, tag="zf")
                        nc.gpsimd.dma_start(zf[0:64, :], zu[h * 64 : h * 64 + 64, 0:HALF])
                        nc.gpsimd.dma_start(
                            zf[64:128, :], zu[h * 64 : h * 64 + 64, HALF:SIZE]
                        )
                        zf_tiles[t] = zf

            # lead: LEAD supertiles (z for steps 0..2*LEAD-1) before the scan;
            # all h0 halves first (they only need gt cols 0-1024, which land
            # before cols 1024-2048 in the DMA queue)
            for st0 in range(min(LEAD, n_st)):
                noise_half(st0, 0)
            for st0 in range(min(LEAD, n_st)):
                noise_half(st0, 1)

            # ---- the scan ----
            for t in range(steps):
                zf = zf_tiles.pop(t)
                # w = (1-dt)*x + z_t, split per chunk: each half depends only
                # on that chunk's add from the previous step, so the c0 half
                # schedules well before the critical c1 add->nl chain
                for c0, c1 in CHUNKS:
                    nc.vector.scalar_tensor_tensor(
                        w[:, c0:c1], x[:, c0:c1], A_COEF, zf[:, c0:c1],
                        OP.mult, OP.add,
                    )
                f16t = f16p.tile([128, HALF], f16, tag="f16t")
                for c0, c1 in CHUNKS:
                    wd = c1 - c0
                    ps = spsp.tile([128, 512], f32, tag="sps")
                    for idx, k in enumerate(range(KT)):
                        fTk = fT_ap(fT_cur, k)
                        nc.tensor.matmul(
                            ps[0:64, :wd],
                            fTk,
                            bt[:, k, c0:c1],
                            start=(idx == 0),
                            stop=(idx == KT - 1),
                            tile_position=(0, 0),
                            skip_group_check=True,
                        )
                        nc.tensor.matmul(
                            ps[64:128, :wd],
                            fTk,
                            bt[:, k, HALF + c0 : HALF + c1],
                            start=(idx == 0),
                            stop=(idx == KT - 1),
                            tile_position=(0, 64),
                            skip_group_check=True,
                        )
                    # x_new = y + w
                    nc.vector.tensor_add(x[:, c0:c1], ps[:, :wd], w[:, c0:c1])
                    nl_chunk(x[:, c0:c1], f16t[:, c0:c1], wd)
                fT_cur = transposes(f16t)
                # interleave noise production at consumption rate:
                # half a supertile after every scan step, LEAD supertiles ahead
                st, half = t // 2 + LEAD, t % 2
                if st < n_st:
                    noise_half(st, half)

            nc.scalar.dma_start(out_d[:], x[:])

    nc.compile()
    return nc


def _prepare_host(x, A, G, noise, steps):
    """Host-side O(n^2) weight prep + per-core input shards."""
    import ml_dtypes

    E4M3 = ml_dtypes.float8_e4m3

    block = SIZE // N_CLASSES
    P = np.zeros((SIZE, SIZE), dtype=np.float32)
    for c in range(N_CLASSES):
        P[c * block : (c + 1) * block, c * block : (c + 1) * block] = 1.0 / block
    Ab = A.reshape(SIZE, N_CLASSES, block).mean(axis=2)
    A_P = np.repeat(Ab, block, axis=1)  # A @ P
    M0 = A - A_P + EIG * P  # A @ (I-P) + P
    bt_np = np.ascontiguousarray((DT * BETA) * M0.T).astype(np.float16)
    gt_np = np.ascontiguousarray((S_GT * math.sqrt(DT) * EPSILON) * G.T).astype(E4M3)

    in_maps = []
    for c in range(N_CORES):
        xs = x[c * BPC : (c + 1) * BPC]
        x0f = np.concatenate([xs[:, :HALF], xs[:, HALF:]], axis=0)
        x0f = np.ascontiguousarray(x0f, dtype=np.float32)
        nsh = noise[:steps, c * BPC : (c + 1) * BPC, :].reshape(steps * BPC, SIZE)
        nt_np = np.ascontiguousarray(nsh.astype(E4M3).T)
        in_maps.append({"x0": x0f, "bt": bt_np, "gt": gt_np, "nt": nt_np})
    return in_maps


def _run(in_maps, steps, trace=False):
    from concourse.bass_utils import run_bass_kernel_spmd

    key = steps
    if key not in _cache:
        _cache[key] = _build(steps)
    nc = _cache[key]
    res = run_bass_kernel_spmd(nc, in_maps, list(range(N_CORES)), trace=trace)
    outs = []
    for c in range(N_CORES):
        of = res.results[c]["out"]
        outs.append(np.concatenate([of[0:64, :], of[64:128, :]], axis=1))
    return np.concatenate(outs, axis=0).astype(np.float32), res


def kernel(x, A, G, noise):
    x = np.asarray(x, dtype=np.float32)
    A = np.asarray(A, dtype=np.float32)
    G = np.asarray(G, dtype=np.float32)
    noise = np.asarray(noise, dtype=np.float32)
    in_maps = _prepare_host(x, A, G, noise, STEPS)
    out, _ = _run(in_maps, STEPS)
    return out
